# revision 34
# baseline (speedup 1.0000x reference)
"""MergeAttentionSubBlockFull on 8 TRN2 NeuronCores (Bass/Tile).

Math (reference):
  xn   = LayerNorm(x) * gamma + beta                       [B,T,NE]
  W_f  = U @ blockdiag(W_in).T @ M_qkv ;  b_f = b_in @ M_qkv
  qkv  = xn @ W_f + b_f ; attention over H heads
  out  = (o @ U).reshape per-model @ W_out_m.T + b_out

Kernel split (host/device):
  * ALL weight folding runs on HOST in fp64 at program-build time and is
    baked into the NEFF as inline constants (the fold is pure weight
    preprocessing; inline constants are DMA'd to HBM once at model load,
    never per-exec).  The device program has x as its ONLY per-exec
    input, no collectives, and runs pure batch-parallel (8 batches per
    core).  The program cache is keyed on a hash of the weight bytes and
    rebuilds if the weights change.
  * device per batch: LayerNorm, bf16 3-pass qk projection, bf16 v,
    attention, fused unmerge+out-proj GEMM (o @ U2 + b_out) with
    U2[:, m*E:(m+1)*E] = U_m @ W_out_m.T.

Precision:
  * score-critical path (qk projection, QK^T) runs as bf16 3-pass
    matmuls: x = a1 + a2 exactly (a1 = bf16(x), a2 = bf16(x - a1));
    A@B ~= a1@b1 + a2@b1 + a1@b2, residual ~2^-18 — fp32-grade logits
    at 1 cycle/row (fp32 matmul costs 4 cycles/row on TRN2).  The host
    fold is fp64-exact before the bf16 split.
  * value path (v, att, att@v, o@U2) is plain bf16 (1 pass).

Host-interface cost model (axon tunnel, measured):
  * ~74 ms fixed floor per exec, regardless of core count
  * ~1.0 ms per MB-per-core of ExternalInput per exec (even when
    device-resident), ~1.5 ms fixed per tensor binding, ~0.75 us per
    instruction, ~10-20 ms one-time collective rendezvous
  * hence: weights as inline consts, one input (x), bf16 output,
    zero collectives.
"""

import hashlib

import numpy as np

import concourse.bacc as bacc
import concourse.mybir as mybir
import concourse.tile as tile

F32 = mybir.dt.float32
BF16 = mybir.dt.bfloat16
AF = mybir.ActivationFunctionType
ALU = mybir.AluOpType

B, T, NE, E, NM, H = 64, 256, 768, 768, 3, 12
HD = NE // H                      # 64
NCORES = 8
BB = B // NCORES                  # 8 batches per core
TOK = BB * T                      # 2048 tokens per core
TE = NM * E                       # 2304
NCH = NE // 128                   # 6 feature chunks
PT = 2 * T                        # tokens per batch-pair
OUT_BF16 = True


def build_program(wq1_np, wq2_np, wk1_np, wk2_np, wfv_np, u2_np, smalls_np):
    # no collectives and no core-dependent logic -> drop the partition_id
    # input (one fewer per-exec tensor binding)
    nc = bacc.Bacc("TRN2", target_bir_lowering=False, debug=False,
                   enable_partition_id=False)

    # ---------------- DRAM I/O ----------------
    x_part = nc.dram_tensor("x_part", [TOK, NE], F32, kind="ExternalInput")
    out_part = nc.dram_tensor("out_part", [TOK, TE],
                              BF16 if OUT_BF16 else F32,
                              kind="ExternalOutput")

    ident_np = np.eye(128, dtype=np.float32)
    identb_dram = nc.inline_tensor(ident_np.astype(mybir.dt.np(BF16)),
                                   name="ident_bf16")
    ones_dram = nc.inline_tensor(np.ones((1, 128), np.float32),
                                 name="ones_row")
    # smalls row: [0:2NE] b_fold qk | [2NE:3NE] b_fold v | [3NE:3NE+TE] b_out
    smalls_d = nc.inline_tensor(smalls_np, name="smalls_c")
    wq1_d = nc.inline_tensor(wq1_np, name="wq1_c")     # [NE, NE] bf16
    wq2_d = nc.inline_tensor(wq2_np, name="wq2_c")
    wk1_d = nc.inline_tensor(wk1_np, name="wk1_c")
    wk2_d = nc.inline_tensor(wk2_np, name="wk2_c")
    wfv_d = nc.inline_tensor(wfv_np, name="wfv_c")     # [NE, E] bf16
    u2_d = nc.inline_tensor(u2_np, name="u2_c")        # [NE, TE] bf16

    with tile.TileContext(nc) as tc:
        with tc.tile_pool(name="persist", bufs=1) as pp, \
             tc.tile_pool(name="xt_p", bufs=2) as xtp, \
             tc.tile_pool(name="stat_p", bufs=4) as stp, \
             tc.tile_pool(name="z_p", bufs=2) as zp, \
             tc.tile_pool(name="xnt_p", bufs=3) as xnp:
            hoist = {"xtp": xtp, "stp": stp, "zp": zp, "xnp": xnp, "pp": pp}
            identb = pp.tile([128, 128], BF16, name="identb")
            hoist["identb"] = identb
            ones1 = pp.tile([1, 128], F32, name="ones1")
            nc.sync.dma_start(identb[:], identb_dram[:])
            nc.sync.dma_start(ones1[:], ones_dram[:])

            bfold = pp.tile([128, 12], F32, name="bfold")
            nc.sync.dma_start(
                bfold[:],
                smalls_d[0:1, 0:2 * NE].rearrange("() (c p) -> p c", p=128))

            # persistent folded weights, loaded from inline constants
            wq1 = [pp.tile([128, NE], BF16, name=f"wq1_{c}")
                   for c in range(NCH)]
            wq2 = [pp.tile([128, NE], BF16, name=f"wq2_{c}")
                   for c in range(NCH)]
            wk1 = [pp.tile([128, NE], BF16, name=f"wk1_{c}")
                   for c in range(NCH)]
            wk2 = [pp.tile([128, NE], BF16, name=f"wk2_{c}")
                   for c in range(NCH)]
            wfv = [pp.tile([128, E], BF16, name=f"wfv{c}") for c in range(NCH)]
            u2_sb = [pp.tile([128, TE], BF16, name=f"u2sb{c}")
                     for c in range(NCH)]
            for c in range(NCH):
                csl = slice(c * 128, (c + 1) * 128)
                nc.sync.dma_start(wq1[c][:], wq1_d[csl, :])
                nc.sync.dma_start(wq2[c][:], wq2_d[csl, :])
                nc.sync.dma_start(wk1[c][:], wk1_d[csl, :])
                nc.sync.dma_start(wk2[c][:], wk2_d[csl, :])
                nc.sync.dma_start(wfv[c][:], wfv_d[csl, :])
                nc.sync.dma_start(u2_sb[c][:], u2_d[csl, :])
            ob_bc = pp.tile([128, TE], F32, name="ob_bc")
            vb_bc = pp.tile([128, E], F32, name="vb_bc")

            # ---- bias row broadcasts across partitions ----
            with nc.named_scope("prep"), \
                 tc.tile_pool(name="p1_sb", bufs=1) as p1p, \
                 tc.tile_pool(name="ps1", bufs=1, space="PSUM") as ps1:
                bout_sb = p1p.tile([1, TE], F32, name="bout_sb")
                nc.sync.dma_start(bout_sb[:],
                                  smalls_d[0:1, 3 * NE:3 * NE + TE])
                bvr_sb = p1p.tile([1, NE], F32, name="bvr_sb")
                nc.sync.dma_start(bvr_sb[:], smalls_d[0:1, 2 * NE:3 * NE])
                for i, w in enumerate([512, 512, 512, 512, 256]):
                    bb_ps = ps1.tile([128, 512], F32, name="bb_ps", tag="bbps",
                                     bufs=2)
                    nc.tensor.matmul(bb_ps[:, :w], ones1[:],
                                     bout_sb[:, i * 512:i * 512 + w],
                                     start=True, stop=True)
                    nc.any.tensor_copy(ob_bc[:, i * 512:i * 512 + w],
                                       bb_ps[:, :w])
                for i, w in enumerate([512, 256]):
                    bb_ps = ps1.tile([128, 512], F32, name="bb_ps", tag="bbps",
                                     bufs=2)
                    nc.tensor.matmul(bb_ps[:, :w], ones1[:],
                                     bvr_sb[:, i * 512:i * 512 + w],
                                     start=True, stop=True)
                    nc.any.tensor_copy(vb_bc[:, i * 512:i * 512 + w],
                                       bb_ps[:, :w])

            _emit_batches(nc, tc, identb, x_part, out_part,
                          wq1, wq2, wk1, wk2, wfv, u2_sb, bfold, ob_bc,
                          vb_bc, hoist)

    nc.compile()
    return nc


def _emit_ln_xnt(nc, hoist, pr, x_part, identb, psum_pool):
    """LayerNorm + bf16 a1/a2 split + transpose for one batch-pair.

    Returns (xnt1, xnt2): 6 chunks each of [128, PT] bf16 (feature-major).
    """
    xtp, stp, zp, xnp = (hoist["xtp"], hoist["stp"], hoist["zp"],
                         hoist["xnp"])
    xnt1 = [xnp.tile([128, PT], BF16, name=f"xnt1_{c}") for c in range(NCH)]
    xnt2 = [xnp.tile([128, PT], BF16, name=f"xnt2_{c}") for c in range(NCH)]
    for i in range(4):
        xt = xtp.tile([128, NE], F32, name="xt")
        nc.sync.dma_start(
            xt[:], x_part[pr * PT + i * 128:pr * PT + (i + 1) * 128, :])
        ssum = stp.tile([128, 1], F32, name="ssum")
        nc.vector.tensor_reduce(ssum[:], xt[:], mybir.AxisListType.X, ALU.add)
        nmu = stp.tile([128, 1], F32, name="nmu")
        nc.vector.tensor_scalar_mul(nmu[:], ssum[:], -1.0 / NE)
        z = zp.tile([128, NE], F32, name="z")
        sumsq = stp.tile([128, 1], F32, name="sumsq")
        nc.scalar.activation(z[:], xt[:], AF.Square, bias=nmu[:],
                             scale=1.0, accum_out=sumsq[:])
        var = stp.tile([128, 1], F32, name="var")
        nc.vector.tensor_scalar(var[:], sumsq[:], 1.0 / NE, 1e-5,
                                ALU.mult, ALU.add)
        std = stp.tile([128, 1], F32, name="std")
        nc.scalar.activation(std[:], var[:], AF.Sqrt)
        rstd = stp.tile([128, 1], F32, name="rstd")
        nc.vector.reciprocal(rstd[:], std[:])
        nmrs = stp.tile([128, 1], F32, name="nmrs")
        nc.vector.tensor_mul(nmrs[:], nmu[:], rstd[:])
        nc.scalar.activation(z[:], xt[:], AF.Identity,
                             bias=nmrs[:], scale=rstd[:])
        z1 = zp.tile([128, NE], BF16, name="z1")
        nc.gpsimd.tensor_copy(z1[:], z[:])
        z2 = zp.tile([128, NE], BF16, name="z2")
        nc.vector.tensor_tensor(z2[:], z[:], z1[:], ALU.subtract)
        for c in range(NCH):
            t_ps = psum_pool.tile([128, 256], F32, name="t_ps",
                                  tag="tps", bufs=2)
            nc.tensor.matmul(t_ps[:, 0:128], z1[:, c * 128:(c + 1) * 128],
                             identb[:], start=True, stop=True)
            nc.tensor.matmul(t_ps[:, 128:256], z2[:, c * 128:(c + 1) * 128],
                             identb[:], start=True, stop=True)
            nc.any.tensor_copy(xnt1[c][:, i * 128:(i + 1) * 128],
                               t_ps[:, 0:128])
            nc.any.tensor_copy(xnt2[c][:, i * 128:(i + 1) * 128],
                               t_ps[:, 128:256])
    return xnt1, xnt2


def _emit_batches(nc, tc, identb, x_part, out_part,
                  wq1, wq2, wk1, wk2, wfv, u2_sb, bfold, ob_bc,
                  vb_bc, hoist):
    HP = H // 2   # head pairs
    stp = hoist["stp"]
    with tc.tile_pool(name="qk_p", bufs=1) as qkp, \
         tc.tile_pool(name="att_p", bufs=2) as atp, \
         tc.tile_pool(name="ot_p", bufs=1) as otp, \
         tc.tile_pool(name="out_p", bufs=2) as outp, \
         tc.tile_pool(name="bps", bufs=1, space="PSUM") as bps:

        def emit_proj(j, w1set, w2set, xnt1, xnt2, out1, out2):
            """One 128-wide q or k chunk, bf16 3-pass + bias + a1/a2 split."""
            q_ps = bps.tile([128, PT], F32, name="q_ps", tag="qo", bufs=2)
            jj = j % 6
            for c in range(NCH):
                nc.tensor.matmul(q_ps[:], w1set[c][:, jj * 128:(jj + 1) * 128],
                                 xnt1[c][:], start=(c == 0), stop=False)
            for c in range(NCH):
                nc.tensor.matmul(q_ps[:], w2set[c][:, jj * 128:(jj + 1) * 128],
                                 xnt1[c][:], start=False, stop=False)
            for c in range(NCH):
                nc.tensor.matmul(q_ps[:], w1set[c][:, jj * 128:(jj + 1) * 128],
                                 xnt2[c][:], start=False, stop=(c == NCH - 1))
            qf = qkp.tile([128, PT], F32, name="qf", tag="qf", bufs=2)
            nc.scalar.activation(qf[:], q_ps[:], AF.Identity,
                                 bias=bfold[:, j:j + 1])
            nc.gpsimd.tensor_copy(out1[:], qf[:])
            nc.vector.tensor_tensor(out2[:], qf[:], out1[:], ALU.subtract)

        for pr in range(BB // 2):
            with nc.named_scope(f"pair{pr}"):
                xnt1, xnt2 = _emit_ln_xnt(nc, hoist, pr, x_part,
                                          identb, bps)

                # ---- v in [token, feature] layout, bf16 (4 tok chunks) ----
                v_t = [qkp.tile([128, E], BF16, name=f"vt{i}", bufs=2)
                       for i in range(4)]
                for i in range(4):
                    for s0, w in [(0, 512), (512, 256)]:
                        v_ps = bps.tile([128, 512], F32, name="v_ps",
                                        tag="qo", bufs=2)
                        for c in range(NCH):
                            nc.tensor.matmul(
                                v_ps[:, 0:w],
                                xnt1[c][:, i * 128:(i + 1) * 128],
                                wfv[c][:, s0:s0 + w],
                                start=(c == 0), stop=(c == NCH - 1))
                        nc.vector.tensor_add(v_t[i][:, s0:s0 + w],
                                             v_ps[:, 0:w],
                                             vb_bc[:, s0:s0 + w])

                # ---- q + k projections ----
                qk1 = [qkp.tile([128, PT], BF16, name=f"qk1_{j}")
                       for j in range(12)]
                qk2 = [qkp.tile([128, PT], BF16, name=f"qk2_{j}")
                       for j in range(12)]
                for j in range(6):
                    emit_proj(j, wq1, wq2, xnt1, xnt2, qk1[j], qk2[j])
                for j in range(6):
                    emit_proj(6 + j, wk1, wk2, xnt1, xnt2, qk1[6 + j],
                              qk2[6 + j])

                # ---- attention + out GEMM per batch in the pair ----
                for bl in range(2):
                    b0 = bl * T
                    ot_sb = [otp.tile([128, T], BF16, name=f"ot{hp}")
                             for hp in range(HP)]
                    for hp in range(HP):
                        att_bf = {}
                        for qc in range(2):
                            s_ps = [bps.tile([128, T], F32, name=f"s_ps{hh}",
                                             tag="sps", bufs=3)
                                    for hh in range(2)]
                            for hh in range(2):
                                r0 = hh * 64
                                qsl = slice(b0 + qc * 128, b0 + (qc + 1) * 128)
                                ksl = slice(b0, b0 + T)
                                nc.tensor.matmul(
                                    s_ps[hh][:],
                                    qk1[hp][r0:r0 + 64, qsl],
                                    qk1[6 + hp][r0:r0 + 64, ksl],
                                    start=True, stop=False,
                                    tile_position=(r0, 0))
                                nc.tensor.matmul(
                                    s_ps[hh][:],
                                    qk2[hp][r0:r0 + 64, qsl],
                                    qk1[6 + hp][r0:r0 + 64, ksl],
                                    start=False, stop=False,
                                    tile_position=(r0, 0))
                                nc.tensor.matmul(
                                    s_ps[hh][:],
                                    qk1[hp][r0:r0 + 64, qsl],
                                    qk2[6 + hp][r0:r0 + 64, ksl],
                                    start=False, stop=True,
                                    tile_position=(r0, 0))
                            for hh in range(2):
                                nmax = stp.tile([128, 1], F32, name="nmax")
                                nc.vector.tensor_reduce(nmax[:], s_ps[hh][:],
                                                        mybir.AxisListType.X,
                                                        ALU.max, negate=True)
                                att = atp.tile([128, T], BF16, name="att",
                                               bufs=3)
                                sm = stp.tile([128, 1], F32, name="sm")
                                nc.scalar.activation(att[:], s_ps[hh][:],
                                                     AF.Exp, bias=nmax[:],
                                                     accum_out=sm[:])
                                rs = stp.tile([128, 1], F32, name="rs")
                                nc.vector.reciprocal(rs[:], sm[:])
                                abf = atp.tile([128, T], BF16, name="abf",
                                               bufs=4)
                                nc.gpsimd.tensor_scalar_mul(abf[:], att[:],
                                                            rs[:])
                                att_bf[(hh, qc)] = abf
                        o_ps = bps.tile([128, T], F32, name="o_ps", tag="ops",
                                        bufs=1)
                        for hh in range(2):
                            attT = [atp.tile([128, T], BF16, name=f"attT{kc}")
                                    for kc in range(2)]
                            for kc in range(2):
                                tr_ps = bps.tile([128, 256], F32,
                                                 name="t_ps", tag="tps",
                                                 bufs=2)
                                for qc in range(2):
                                    nc.tensor.matmul(
                                        tr_ps[:, qc * 128:(qc + 1) * 128],
                                        att_bf[(hh, qc)][:, kc * 128:(kc + 1) * 128],
                                        identb[:], start=True, stop=True)
                                nc.any.tensor_copy(attT[kc][:], tr_ps[:])
                            r0 = hh * 64
                            h = 2 * hp + hh
                            for kc in range(2):
                                nc.tensor.matmul(
                                    o_ps[r0:r0 + 64, :],
                                    v_t[bl * 2 + kc][:, h * HD:(h + 1) * HD],
                                    attT[kc][:],
                                    start=(kc == 0), stop=(kc == 1),
                                    tile_position=(0, r0))
                        nc.any.tensor_copy(ot_sb[hp][:], o_ps[:])

                    # ---- out = oT.T @ U2 + b_out ----
                    for tc_ in range(2):
                        ou = outp.tile([128, TE],
                                       BF16 if OUT_BF16 else F32, name="ou")
                        for noc, w in enumerate([512, 512, 512, 512, 256]):
                            oo_ps = bps.tile([128, 512], F32, name="oo_ps",
                                             tag="qo", bufs=2)
                            for c in range(NCH):
                                nc.tensor.matmul(
                                    oo_ps[:, 0:w],
                                    ot_sb[c][:, tc_ * 128:(tc_ + 1) * 128],
                                    u2_sb[c][:, noc * 512:noc * 512 + w],
                                    start=(c == 0), stop=(c == NCH - 1))
                            nc.vector.tensor_add(
                                ou[:, noc * 512:noc * 512 + w],
                                oo_ps[:, 0:w],
                                ob_bc[:, noc * 512:noc * 512 + w])
                        nc.sync.dma_start(
                            out_part[(pr * 2 + bl) * T + tc_ * 128:
                                     (pr * 2 + bl) * T + (tc_ + 1) * 128, :],
                            ou[:])


_CACHE = {}


class _Results:
    def __init__(self, results):
        self.results = results


def run_bass_kernel_spmd(nc, in_maps, core_ids, **kwargs):
    """Cached PJRT executor (drop-in for bass_utils.run_bass_kernel_spmd).

    Builds the jitted shard_map wrapper once per program and reuses it
    across calls, so repeated kernel() invocations skip jit re-tracing.
    """
    import jax
    from jax.experimental.shard_map import shard_map
    from jax.sharding import Mesh, PartitionSpec

    from concourse import bass2jax

    n_cores = len(core_ids)
    ec = _CACHE.get("exec")
    if ec is None or ec["nc"] is not nc:
        bass2jax.install_neuronx_cc_hook()
        partition_name = (nc.partition_id_tensor.name
                          if nc.partition_id_tensor else None)
        in_names, out_names, out_avals, out_shapes = [], [], [], []
        for alloc in nc.m.functions[0].allocations:
            if not isinstance(alloc, mybir.MemoryLocationSet):
                continue
            name = alloc.memorylocations[0].name
            if alloc.kind == "ExternalInput":
                if name != partition_name:
                    in_names.append(name)
            elif alloc.kind == "ExternalOutput":
                shape = tuple(alloc.tensor_shape)
                dtype = mybir.dt.np(alloc.dtype)
                out_names.append(name)
                out_avals.append(jax.core.ShapedArray(shape, dtype))
                out_shapes.append((shape, dtype))
        n_params = len(in_names)
        n_outs = len(out_avals)
        all_in = in_names + out_names + ([partition_name]
                                         if partition_name else [])

        def _body(*args):
            operands = list(args)
            if partition_name is not None:
                operands.append(bass2jax.partition_id_tensor())
            outs = bass2jax._bass_exec_p.bind(
                *operands, out_avals=tuple(out_avals),
                in_names=tuple(all_in), out_names=tuple(out_names),
                lowering_input_output_aliases=(),
                sim_require_finite=True, sim_require_nnan=True, nc=nc)
            return tuple(outs)

        try:
            devices = jax.devices("neuron")[:n_cores]
        except RuntimeError:
            devices = jax.devices()[:n_cores]
        mesh = Mesh(np.asarray(devices), ("core",))
        in_specs = (PartitionSpec("core"),) * (n_params + n_outs)
        out_specs = (PartitionSpec("core"),) * n_outs
        donate = tuple(range(n_params, n_params + n_outs))
        fn = jax.jit(shard_map(_body, mesh=mesh, in_specs=in_specs,
                               out_specs=out_specs, check_rep=False),
                     donate_argnums=donate, keep_unused=True)
        from jax.sharding import NamedSharding
        shard = NamedSharding(mesh, PartitionSpec("core"))
        ec = {"nc": nc, "fn": fn, "in_names": in_names,
              "out_names": out_names, "out_shapes": out_shapes,
              "n_cores": n_cores, "shard": shard, "outs": None}
        _CACHE["exec"] = ec

    fn = ec["fn"]
    shard = ec["shard"]
    concat_in = [
        np.concatenate([np.asarray(in_maps[c][nm]) for c in range(n_cores)],
                       axis=0)
        for nm in ec["in_names"]]
    dev_in = [jax.device_put(a, shard) for a in concat_in]
    if ec["outs"] is None:
        # kernel writes every output element, so initial contents are
        # irrelevant; later calls recycle the donated output buffers.
        obuf = [jax.device_put(np.zeros((n_cores * s[0], *s[1:]), d), shard)
                for s, d in ec["out_shapes"]]
    else:
        obuf = ec["outs"]
    outs = fn(*dev_in, *obuf)
    ec["outs"] = outs
    np_outs = [np.asarray(o) for o in outs]
    results = []
    for c in range(n_cores):
        row = {}
        for nm, o, (s, d) in zip(ec["out_names"], np_outs, ec["out_shapes"]):
            row[nm] = o[c * s[0]:(c + 1) * s[0]]
        results.append(row)
    return _Results(results)


def _bf16(x):
    x = np.asarray(x, np.float32)
    i = x.view(np.uint32)
    return (((i + 0x7FFF + ((i >> 16) & 1)) & 0xFFFF0000).astype(np.uint32)
            ).view(np.float32)


def _split12(x):
    a1 = _bf16(x)
    a2 = _bf16(np.asarray(x, np.float32) - a1)
    return a1, a2


def _to_bf16_bits(x):
    """fp32 -> bf16 numpy array (ml_dtypes bfloat16)."""
    return np.asarray(x, np.float32).astype(mybir.dt.np(BF16))


def _fold_host(ln_gamma, ln_beta, in_proj_weight, in_proj_bias,
               out_proj_weight, out_proj_bias, U, M_qkv):
    """Exact fp64 weight fold; returns the device constant arrays."""
    g = ln_gamma.astype(np.float64)
    beta = ln_beta.astype(np.float64)
    W = in_proj_weight.astype(np.float64)      # [NM, 3E, E]
    bin_ = in_proj_bias.astype(np.float64)     # [NM, 3E]
    Wo = out_proj_weight.astype(np.float64)    # [NM, E, E]
    bo = out_proj_bias.astype(np.float64)      # [NM, E]
    U64 = U.astype(np.float64)                 # [NE, NM*E]

    qsv = np.ones(3 * NE, np.float64)
    qsv[:NE] = 1.0 / np.sqrt(HD)               # fold 1/sqrt(hd) into M
    M64 = M_qkv.astype(np.float64) * qsv       # [NM*3E, 3NE]

    # W_f = U @ blockdiag(W).T @ M  (without materializing the blockdiag)
    U_r = U64.reshape(NE, NM, E)
    UWt = np.einsum('nme,moe->nmo', U_r, W)    # [NE, NM, 3E]
    W_f = UWt.reshape(NE, NM * 3 * E) @ M64    # [NE, 3NE]

    # b_f = b_in @ M + (beta @ U @ blockdiag(W).T) @ M  (LN beta folded)
    bU = beta @ U64                            # [NM*E]
    bUW = np.concatenate(
        [bU[m * E:(m + 1) * E] @ W[m].T for m in range(NM)])
    b_f = bin_.reshape(-1) @ M64 + bUW @ M64   # [3NE]

    W_fg = g[:, None] * W_f                    # LN gamma folded

    # U2[:, m*E:(m+1)*E] = U_m @ W_out_m.T  (unmerge+out-proj fused)
    U2 = np.concatenate(
        [U64[:, m * E:(m + 1) * E] @ Wo[m].T for m in range(NM)],
        axis=1)                                # [NE, TE]

    wq1_np, wq2_np = (_to_bf16_bits(a) for a in _split12(W_fg[:, 0:NE]))
    wk1_np, wk2_np = (_to_bf16_bits(a) for a in _split12(W_fg[:, NE:2 * NE]))
    wfv_np = _to_bf16_bits(W_fg[:, 2 * NE:])
    u2_np = _to_bf16_bits(U2)
    smalls_np = np.concatenate(
        [b_f, bo.reshape(-1)]).astype(np.float32).reshape(1, 3 * NE + TE)
    return wq1_np, wq2_np, wk1_np, wk2_np, wfv_np, u2_np, smalls_np


def kernel(x, ln_gamma, ln_beta, in_proj_weight, in_proj_bias,
           out_proj_weight, out_proj_bias, U, M_qkv, num_heads):
    x = np.asarray(x, np.float32)
    ln_gamma = np.asarray(ln_gamma, np.float32)
    ln_beta = np.asarray(ln_beta, np.float32)
    in_proj_weight = np.asarray(in_proj_weight, np.float32)
    in_proj_bias = np.asarray(in_proj_bias, np.float32)
    out_proj_weight = np.asarray(out_proj_weight, np.float32)
    out_proj_bias = np.asarray(out_proj_bias, np.float32)
    U = np.asarray(U, np.float32)
    M_qkv = np.asarray(M_qkv, np.float32)
    assert int(num_heads) == H

    hsh = hashlib.blake2b(digest_size=16)
    for a in (ln_gamma, ln_beta, in_proj_weight, in_proj_bias,
              out_proj_weight, out_proj_bias, U, M_qkv):
        hsh.update(np.ascontiguousarray(a).tobytes())
    key = hsh.hexdigest()
    if _CACHE.get("key") != key:
        consts = _fold_host(ln_gamma, ln_beta, in_proj_weight, in_proj_bias,
                            out_proj_weight, out_proj_bias, U, M_qkv)
        _CACHE["nc"] = build_program(*consts)
        _CACHE["key"] = key
    nc = _CACHE["nc"]

    in_maps = [
        {"x_part": np.ascontiguousarray(
            x[core * BB:(core + 1) * BB].reshape(TOK, NE))}
        for core in range(NCORES)]

    res = run_bass_kernel_spmd(nc, in_maps, list(range(NCORES)))
    out = np.empty((B, T, TE), np.float32)
    for core in range(NCORES):
        out[core * BB:(core + 1) * BB] = \
            res.results[core]["out_part"].astype(np.float32).reshape(BB, T, TE)
    return out
